# revision 14
# baseline (speedup 1.0000x reference)
"""Trainium2 Bass kernel for BlockDecomposedSSMAttention.

Math: y[b,s,:] = x[b,s,:] @ B.T @ A @ C.T   (no cross-block recurrence)
 ==>  y = x @ W  with  W = B.T @ (A @ C.T)

Distribution over the 8 NeuronCores — 2D grid RG x CG (x-rows x W-cols):
  - x is sharded over (batch*seq) into RG row groups (replicated CG times).
  - W is sharded by output columns into CG groups. The right-to-left
    bracketing makes a W column-slice computable from a C row-slice alone:
        G[:, O]  = A  @ C.T[:, O]      (needs full A, C slice)
        W[:, O]  = B.T @ G[:, O]       (needs full B)
    so no core computes more than OCW/1024 of the fold and no collective
    is needed.  vs the 1D baseline this removes ~3/4 of the redundant
    W-fold PE work (the dominant overhead: fold was 50% of all PE cycles).
  - main: y.T[O, rows] tile-wise = W[:,O].T-tiles (stationary) @ x.T
    (moving, N=512), accumulated over the 8 k-tiles in PSUM.

All HBM traffic is bf16 (host casts; PSUM accumulation stays f32), halving
DMA bytes; stage results G/W are quantized to bf16 in SBUF.  Host-side work
is layout marshalling only (shard slicing, transposes, dtype casts); every
FLOP runs on the device.
"""

import os
import sys

import numpy as np

if "/opt/trn_rl_repo" not in sys.path:
    sys.path.insert(0, "/opt/trn_rl_repo")

BATCH, SEQ, D = 4, 4096, 1024
NCORES = 8
RG = 2                         # row groups (x shards)
CG = NCORES // RG              # col groups (W shards)
ROWS = BATCH * SEQ             # 16384
MSH = ROWS // RG               # rows per core
OCW = D // CG                  # W out-cols per core
P = 128
KT = D // P                    # 8 contraction tiles
MC = MSH // 512                # moving chunks of the main loop
OT = OCW // P                  # stationary W tiles per core
GRP = 1                        # mc per main-loop group

_CACHE: dict = {}


def _build_nc():
    import concourse.mybir as mybir
    import concourse.tile as tile
    from concourse import bacc

    f32 = mybir.dt.float32
    bf16 = mybir.dt.bfloat16

    nc = bacc.Bacc(
        "TRN2", target_bir_lowering=False, debug=False, num_devices=NCORES
    )

    # Per-core inputs, host pre-arranged so every DMA is contiguous 2KB+
    # lines and every matmul operand is a natural [contraction-on-
    # partitions] SBUF load:
    #   at: A.T as [kp, ko, h]   (lhsT tiles for G = A @ C.T[:,O])
    #   ct: C.T[:, O] as [kp, ko, o]           (rhs for G)
    #   b:  B as [hp, ho, i]     (lhsT tiles for W = B.T @ G)
    #   xt: x_shard.T as [ip, io, m]           (rhs for main loop)
    at_in = nc.dram_tensor("at_in", [P, KT, D], bf16, kind="ExternalInput")
    ct_in = nc.dram_tensor("ct_in", [P, KT, OCW], bf16, kind="ExternalInput")
    b_in = nc.dram_tensor("b_in", [P, KT, D], bf16, kind="ExternalInput")
    xt = nc.dram_tensor("xt", [P, KT, MSH], bf16, kind="ExternalInput")
    # y.T shard [OCW, MSH]; host transposes back
    yt_out = nc.dram_tensor("yt_out", [OCW, MSH], bf16, kind="ExternalOutput")

    with tile.TileContext(nc) as tc:
        with (
            tc.tile_pool(name="ycopy", bufs=6) as ycopy,
            tc.tile_pool(name="ps8", bufs=8, space="PSUM") as ps8,
        ):
            at_sb, _free_at = tc.tile([P, KT, D], bf16, name="at_sb")
            ct_sb, _free_ct = tc.tile([P, KT, OCW], bf16, name="ct_sb")
            b_sb, _free_b = tc.tile([P, KT, D], bf16, name="b_sb")
            xt_sb, _free_x = tc.tile([P, KT, MSH], bf16, name="xt_sb")
            g_sb, _free_g = tc.tile([P, KT, OCW], bf16, name="g_sb")
            w_sb, _free_w = tc.tile([P, KT, OCW], bf16, name="w_sb")

            # ---- input DMA, one queue (sync), in consumption order ----
            # (each dma_start costs ~0.6us of queue-engine issue time;
            # chunks sized/ordered so stage G streams just-in-time: G group
            # htg only reads at[:, ko, htg*512:(htg+1)*512])
            nc.sync.dma_start(ct_sb[:, 0, :], ct_in.ap()[:, 0, :])
            nc.sync.dma_start(at_sb[:, 0, :], at_in.ap()[:, 0, :])
            nc.sync.dma_start(ct_sb[:, 1:KT, :], ct_in.ap()[:, 1:KT, :])
            for ko in range(1, KT):
                nc.sync.dma_start(at_sb[:, ko, :], at_in.ap()[:, ko, :])
            # x chunk 0 ahead of b: main group 0 needs it right as the W
            # stage drains; b rows land just-in-time for W's ho-loop.
            XCH = 512
            nc.sync.dma_start(xt_sb[:, :, 0:XCH], xt.ap()[:, :, 0:XCH])
            for hq in range(4):
                nc.sync.dma_start(
                    b_sb[:, hq * 2 : (hq + 1) * 2, :],
                    b_in.ap()[:, hq * 2 : (hq + 1) * 2, :],
                )
            for mq in range(1, MSH // XCH):
                nc.sync.dma_start(
                    xt_sb[:, :, mq * XCH : (mq + 1) * XCH],
                    xt.ap()[:, :, mq * XCH : (mq + 1) * XCH],
                )

            # PSUM: 8 bank-sized slots cycled by name; stages use 4 at a
            # time so the next group overlaps the previous group's copies.
            def pbank():
                t = ps8.tile([P, 512], f32, name="bank")
                return t

            # ---- PE warm-up during the DMA dead-time: ~8 matmuls on a
            # memset tile flip the HAM clock gate (1.2->2.4GHz) before the
            # first real matmul; result is never read.
            wu_sb, _free_wu = tc.tile([P, 512], bf16, name="wu_sb")
            nc.gpsimd.memset(wu_sb[:], 0.0)
            wu_ps = pbank()
            for r in range(2):
                nc.tensor.matmul(
                    wu_ps[:],
                    wu_sb[:, 0:P],
                    wu_sb[:],
                    start=(r == 0),
                    stop=(r == 1),
                )

            # ---- stage G: G[:, O] = A @ Ct[:, O]  (ko-inner, 4 banks) ----
            for htg in range(KT // 4):
                ps = [pbank() for _ in range(4)]
                for ko in range(KT):
                    for j in range(4):
                        ht = htg * 4 + j
                        nc.tensor.matmul(
                            ps[j][:, 0:OCW],
                            at_sb[:, ko, ht * P : (ht + 1) * P],
                            ct_sb[:, ko, :],
                            start=(ko == 0),
                            stop=(ko == KT - 1),
                        )
                for j in range(4):
                    nc.vector.tensor_copy(g_sb[:, htg * 4 + j, :], ps[j][:, 0:OCW])

            # ---- stage W: W[:, O] = B.T @ G[:, O]  (ho-inner) ----
            for itg in range(KT // 4):
                ps = [pbank() for _ in range(4)]
                for ho in range(KT):
                    for j in range(4):
                        it = itg * 4 + j
                        nc.tensor.matmul(
                            ps[j][:, 0:OCW],
                            b_sb[:, ho, it * P : (it + 1) * P],
                            g_sb[:, ho, :],
                            start=(ho == 0),
                            stop=(ho == KT - 1),
                        )
                for j in range(4):
                    nc.vector.tensor_copy(w_sb[:, itg * 4 + j, :], ps[j][:, 0:OCW])

            # ---- main: y.T[ot*128.., mc*512..] = W-tile.T @ x.T-chunk ----
            # mc ascending matches the x DMA stream order; two single-mc
            # lead groups let the loop start on just 1MB of x.
            groups = [[m] for m in range(MC)]
            for gi, mcs in enumerate(groups):
                ps = {}
                for mc in mcs:
                    for ot in range(OT):
                        ps[(mc, ot)] = pbank()
                for it in range(KT):
                    for ot in range(OT):
                        for mc in mcs:
                            nc.tensor.matmul(
                                ps[(mc, ot)][:],
                                w_sb[:, it, ot * P : (ot + 1) * P],
                                xt_sb[:, it, mc * 512 : (mc + 1) * 512],
                                start=(it == 0),
                                stop=(it == KT - 1),
                            )
                for mc in mcs:
                    for ot in range(OT):
                        yt = ycopy.tile([P, 512], bf16, name="yt")
                        nc.vector.tensor_copy(yt[:], ps[(mc, ot)][:])
                        nc.scalar.dma_start(
                            yt_out.ap()[
                                ot * P : (ot + 1) * P, mc * 512 : (mc + 1) * 512
                            ],
                            yt[:],
                        )

            # release singles LIFO so pool stack unwinds cleanly
            _free_wu()
            _free_w()
            _free_g()
            _free_x()
            _free_b()
            _free_ct()
            _free_at()

    nc.compile()
    return nc


def _get_nc():
    if "nc" not in _CACHE:
        _CACHE["nc"] = _build_nc()
    return _CACHE["nc"]


def _rearr(m, last):
    """[1024, last] row-major -> [128, 8, last] with (kp, ko, :) = row ko*128+kp."""
    return np.ascontiguousarray(m.reshape(KT, P, last).transpose(1, 0, 2))


def _make_in_maps(x, A, B, C):
    import ml_dtypes

    bf = ml_dtypes.bfloat16
    x2 = np.ascontiguousarray(x, dtype=np.float32).reshape(ROWS, D)
    at = _rearr(np.ascontiguousarray(A.T, dtype=np.float32), D).astype(bf)
    b_r = _rearr(np.ascontiguousarray(B, dtype=np.float32), D).astype(bf)
    cts = [
        _rearr(
            np.ascontiguousarray(C[q * OCW : (q + 1) * OCW, :].T, np.float32), OCW
        ).astype(bf)
        for q in range(CG)
    ]
    xts = []
    for m in range(RG):
        shard = x2[m * MSH : (m + 1) * MSH]  # [MSH, D]
        xts.append(
            np.ascontiguousarray(
                shard.reshape(MSH, KT, P).transpose(2, 1, 0)
            ).astype(bf)
        )
    in_maps = []
    for c in range(NCORES):
        m, q = c // CG, c % CG
        in_maps.append({"xt": xts[m], "at_in": at, "b_in": b_r, "ct_in": cts[q]})
    return in_maps


def _install_ntff_hook():
    """The agent image's ``antenv`` lacks ``axon_hooks``; recreate it and
    register the ctypes-based NTFF profile hook (same as trn_boot's
    ``_ntff_profile_via_ctypes``) so ``trace=True`` yields exec_time_ns."""
    import contextlib
    import ctypes
    import types

    if "antenv.axon_hooks" in sys.modules:
        return True
    so_path = "/opt/axon/libaxon_pjrt.so"
    if not os.path.exists(so_path):
        return False
    lib = ctypes.CDLL(so_path)
    if not hasattr(lib, "axon_start_nrt_profile"):
        return False
    lib.axon_start_nrt_profile.argtypes = [
        ctypes.POINTER(ctypes.c_int64),
        ctypes.c_size_t,
    ]
    lib.axon_start_nrt_profile.restype = ctypes.c_int64
    lib.axon_stop_nrt_profile.argtypes = [ctypes.c_char_p]
    lib.axon_stop_nrt_profile.restype = ctypes.c_int64

    @contextlib.contextmanager
    def _hook(output_dir, device_ids):
        import jax

        jax.devices()
        if device_ids:
            ids = (ctypes.c_int64 * len(device_ids))(*device_ids)
            rc = lib.axon_start_nrt_profile(ids, len(device_ids))
        else:
            rc = lib.axon_start_nrt_profile(None, 0)
        if rc != 0:
            raise RuntimeError(f"axon_start_nrt_profile rc={rc}")
        try:
            yield
        finally:
            n = lib.axon_stop_nrt_profile(str(output_dir).encode())
            print(f"ntff profile: {n} file(s) written to {output_dir}")

    mod = types.ModuleType("antenv.axon_hooks")
    _state = {"hook": _hook}
    mod.set_axon_ntff_profile_hook = lambda h: _state.__setitem__("hook", h)
    mod.get_axon_ntff_profile_hook = lambda: _state["hook"]
    sys.modules["antenv.axon_hooks"] = mod
    import antenv

    antenv.axon_hooks = mod
    return True


def run(x, A, B, C, trace=False):
    """Run on hardware; returns (y_full, exec_time_ns_or_None)."""
    from concourse import bass_utils
    from concourse.bass_interp import get_hw_module

    if trace and not _install_ntff_hook():
        trace = False
    if trace:
        # upload_artifacts pushes the NEFF dir to a remote bucket; in this
        # sandbox that can fail AFTER a successful run, losing the results.
        # Degrade to the local path. (Only touches the tracing dev path.)
        if not getattr(bass_utils.upload_artifacts, "_safe", False):
            _orig_upload = bass_utils.upload_artifacts

            def _safe_upload(tmpdir):
                try:
                    return _orig_upload(tmpdir)
                except Exception as e:
                    print(f"upload_artifacts skipped ({type(e).__name__}): {e}")
                    return str(tmpdir)

            _safe_upload._safe = True
            bass_utils.upload_artifacts = _safe_upload

    nc = _get_nc()
    in_maps = _make_in_maps(x, A, B, C)

    old_m = nc.m
    nc.m = get_hw_module(nc.m)
    try:
        res = bass_utils.run_bass_kernel_spmd(
            nc, in_maps, core_ids=list(range(NCORES)), trace=trace
        )
    finally:
        nc.m = old_m

    y = np.empty((ROWS, D), dtype=np.float32)
    for c in range(NCORES):
        m, q = c // CG, c % CG
        yt = np.asarray(res.results[c]["yt_out"])  # [OCW, MSH] bf16
        y[m * MSH : (m + 1) * MSH, q * OCW : (q + 1) * OCW] = yt.T.astype(
            np.float32
        )
    return y.reshape(BATCH, SEQ, D), res.exec_time_ns


def kernel(x, A, B, C):
    y, _ = run(x, A, B, C, trace=False)
    return y
